# revision 33
# baseline (speedup 1.0000x reference)
"""v16: v15 + fp8 DoubleRow for attn@V (key-tile pairs) and QKV
projections (input-channel pairs), with a bf16 exception path for the
first two key tiles (early queries attend to few keys, so fp8 noise
does not average out there).

- Scales: W8{q,k,v} = fp8(16*W); x8 = fp8(x). Raw q/k = 16*(true), so
  raw scores = 256*(true) -> exp scale = 0.125/256, mask = -61440.
  v8 = fp8(16*v) with a 16.0 ones-row, so the softmax denominator
  carries the same 16x and normalization cancels it exactly.
- attn@V: one DoubleRow matmul per pair of key tiles: lhsT = v8 pair
  [128,(2,65)], rhs = pt8 pair [128,(2,F)] (exp writes fp8 halves).
  Pool memsets the causal gap in the odd half of diagonal pairs.
- Exception: (qt=0, pair 0) uses bf16 probs and a bf16 (16x) V for
  tiles 0,1, computed from a small bf16 side-path of the projections.
- Diagonal exp merges both heads via [128,2,F] strided APs.
Baseline v15 measured ~325-343us HW slope here; v16 targets the PE
(scores f32r unchanged; proj+attnV halved) and Act (fewer instrs).
"""

import numpy as np

B = 8
L = 2048
D = 512
H = 8
DH = 64
NT = L // 128
NCH = D // 128
NP = NCH // 2   # ci pairs
NQ = L // 512

_cached = {}


def _walk_instructions(fn):
    insts = []

    def walk(block):
        for i in block.instructions:
            insts.append(i)
            for bb in getattr(i, 'blocks', []) or []:
                walk(bb)
    for bb in fn.blocks:
        walk(bb)
    return insts


def _audit_war(nc):
    """Post-build WAR audit using final PSUM bank / SBUF addr assignment.

    Returns missing ordering edges as (writer_inst_name, user_inst_name):
    the next allocation's writer must wait for the previous same-slot
    allocation's readers/writers.  Used by get_nc's fixpoint loop.
    """
    from collections import defaultdict, OrderedDict
    fn = nc.m.functions[0]
    insts = _walk_instructions(fn)
    idx = {i.name: n for n, i in enumerate(insts)}
    n = len(insts)
    preds = [[] for _ in range(n)]
    for i in insts:
        b = idx[i.name]
        for dn in list(i.sync_dependency_names()) + \
                list(i.nosync_dependency_names()):
            a = idx.get(dn)
            if a is not None:
                preds[b].append(a)
    last_per_engine = {}
    for i in insts:
        if type(i).__name__ in ('InstDMACopy', 'InstTensorLoad',
                                'InstTensorSave'):
            continue
        e = getattr(i, 'engine', None)
        if e is None:
            continue
        b = idx[i.name]
        if e in last_per_engine:
            preds[b].append(last_per_engine[e])
        last_per_engine[e] = b
    anc = [0] * n
    for _ in range(3):
        changed = False
        for b in range(n):
            a_new = anc[b]
            for a in preds[b]:
                a_new |= anc[a] | (1 << a)
            if a_new != anc[b]:
                anc[b] = a_new
                changed = True
        if not changed:
            break
    allocs = OrderedDict()
    for alloc in fn.allocations:
        try:
            ml = alloc.memorylocations[0]
        except Exception:
            continue
        pool = getattr(ml, 'ant_tile_pool_name', None)
        if not pool:
            continue
        if ml.type == 'PSUM':
            key = (pool, 'PSUM', ml.bank)
        else:
            key = (pool, str(ml.type), getattr(ml, 'addr', 0),
                   getattr(ml, 'base', 0))
        allocs.setdefault(key, []).append(str(ml.name))
    readers = defaultdict(list)
    writers = defaultdict(list)
    for i in insts:
        try:
            outs, ins_ = list(i.outs), list(i.ins)
        except Exception:
            continue
        for x in outs:
            m = str(getattr(x, 'memref', '') or '')
            if m:
                writers[m].append(idx[i.name])
        for x in ins_:
            m = str(getattr(x, 'memref', '') or '')
            if m:
                readers[m].append(idx[i.name])
    edges = set()
    for key, names in allocs.items():
        fw = {nm: (min(writers[nm]) if writers[nm] else 1 << 40)
              for nm in names}
        names_sorted = sorted(names, key=lambda nm: fw[nm])
        for prev, nxt in zip(names_sorted, names_sorted[1:]):
            prev_users = readers[prev] + writers[prev]
            for w in writers[nxt]:
                for ru in prev_users:
                    if ru != w and not (anc[w] >> ru) & 1:
                        edges.add((insts[w].name, insts[ru].name))
    return edges


def _add_reuse_guards(tc, nc):
    """Add WAR dependency edges for pool buffer rotation.

    The tile framework does not reliably order a pool allocation's writers
    after the *previous* same-slot allocation's cross-engine readers (e.g.
    PSUM scores matmul restarting a bank the Activation exp still has to
    read).  Walk the emitted instructions, reconstruct the slot rotation
    per (pool, tag) from allocation order, and add edges: first toucher
    per engine of allocation N  depends on  last toucher per engine of
    allocation N-bufs.  Same-engine order needs no edge; DMA copies are
    not collapsed (queue order not guaranteed).
    """
    from concourse.tile_rust import add_dep_helper

    insts = []

    def walk(block):
        for i in block.instructions:
            insts.append(i)
            for bb in getattr(i, 'blocks', []) or []:
                walk(bb)
    assert nc.cur_f is not None
    for bb in nc.cur_f.blocks:
        walk(bb)

    touch = {}
    for pos, i in enumerate(insts):
        try:
            ops = list(i.outs) + list(i.ins)
        except Exception:
            continue
        for x in ops:
            m = getattr(x, 'memref', None)
            if not m:
                bap = getattr(x, 'bass_ap', None)
                if bap is not None:
                    m = getattr(getattr(bap, 'tensor', None), 'name', None)
            if m:
                touch.setdefault(str(m), []).append((pos, i))

    by_tag = {}
    for t in tc.tiles:
        bufs = t.tag_meta.bufs if t.tag_meta is not None else 1
        by_tag.setdefault(t.tag, (bufs, []))[1].append(t.name)

    n_edges = 0
    for tag, (bufs, names) in by_tag.items():
        if len(names) <= bufs:
            continue
        for k in range(bufs, len(names)):
            prev, nxt = names[k - bufs], names[k]
            pu, nu = touch.get(prev, []), touch.get(nxt, [])
            if not pu or not nu:
                continue
            lasts = {}
            for pos, i in pu:
                e = str(i.engine)
                if type(i).__name__ == 'InstDMACopy':
                    lasts[f'dma{pos}'] = i
                else:
                    lasts[e] = i
            firsts = {}
            for pos, i in nu:
                e = str(i.engine)
                if type(i).__name__ == 'InstDMACopy':
                    firsts[f'dma{pos}'] = i
                elif e not in firsts:
                    firsts[e] = i
            for fe, fi in firsts.items():
                for le, li in lasts.items():
                    if fe == le and not fe.startswith('dma'):
                        continue
                    if fi is li:
                        continue
                    add_dep_helper(fi, li, sync=True,
                                   reason=f"pool-reuse WAR guard {tag}")
                    n_edges += 1
    return n_edges


def _build(repeat=1, cfg=None, extra_deps=()):
    cfg = dict(cfg or {})
    PS512 = cfg.get("ps512", 2)
    SPS2 = cfg.get("sps2", 2)
    OPS = cfg.get("ops", 2)
    PEXP = cfg.get("pexp", 6)
    OSB = cfg.get("osb", 3)
    XIN = cfg.get("xin", 8)
    XT = cfg.get("xt", 10)
    import concourse.tile as tile
    from concourse import mybir, bacc
    from concourse.masks import make_identity

    f32 = mybir.dt.float32
    bf16 = mybir.dt.bfloat16
    f32r = mybir.dt.float32r
    f8 = mybir.dt.float8e4
    DR = mybir.MatmulPerfMode.DoubleRow

    nc = bacc.Bacc("TRN2", target_bir_lowering=False, debug=False)

    xq = nc.dram_tensor("query", [L, D], f32, kind="ExternalInput").ap()
    xk = nc.dram_tensor("key", [L, D], f32, kind="ExternalInput").ap()
    xv = nc.dram_tensor("value", [L, D], f32, kind="ExternalInput").ap()
    Wq = nc.dram_tensor("Wq", [D, D], f32, kind="ExternalInput").ap()
    Wk = nc.dram_tensor("Wk", [D, D], f32, kind="ExternalInput").ap()
    Wv = nc.dram_tensor("Wv", [D, D], f32, kind="ExternalInput").ap()
    Wo = nc.dram_tensor("Wo", [D, D], f32, kind="ExternalInput").ap()
    bq = nc.dram_tensor("bq", [D], f32, kind="ExternalInput").ap()
    bk = nc.dram_tensor("bk", [D], f32, kind="ExternalInput").ap()
    bv = nc.dram_tensor("bv", [D], f32, kind="ExternalInput").ap()
    bo = nc.dram_tensor("bo", [D], f32, kind="ExternalInput").ap()
    out = nc.dram_tensor("out", [L, D], f32, kind="ExternalOutput").ap()
    DEBUG = cfg.get("debug", False)
    dbg = {}
    if DEBUG:
        dbg["kt0"] = nc.dram_tensor("dbg_kt0", [128, 512], f32,
                                    kind="ExternalOutput").ap()
        dbg["qt0"] = nc.dram_tensor("dbg_qt0", [128, 512], f32,
                                    kind="ExternalOutput").ap()
        dbg["vbf0"] = nc.dram_tensor("dbg_vbf0", [128, H, DH + 1], bf16,
                                     kind="ExternalOutput").ap()
        dbg["v80"] = nc.dram_tensor("dbg_v80", [128, 2, H, DH + 1],
                                    mybir.dt.float8e4,
                                    kind="ExternalOutput").ap()
        dbg["pt0"] = nc.dram_tensor("dbg_pt0", [128, 2, 2, 512], bf16,
                                    kind="ExternalOutput").ap()
        dbg["po0"] = nc.dram_tensor("dbg_po0", [65, 512], f32,
                                    kind="ExternalOutput").ap()

    def r(ap):
        return ap.bitcast(f32r)

    with tile.TileContext(nc) as tc:
        with (
            tc.tile_pool(name="persist", bufs=1) as persist,
            tc.tile_pool(name="consts", bufs=1) as consts,
            tc.tile_pool(name="ps512", bufs=PS512, space="PSUM") as ps512,
            tc.tile_pool(name="sps2", bufs=SPS2, space="PSUM") as sps2_pool,
            tc.tile_pool(name="ops", bufs=OPS, space="PSUM") as ops_pool,
        ):
            # ---- constants ----
            ident = consts.tile([128, 128], f32, tag="ident")
            make_identity(nc, ident[:])
            # maskU[p, c] = -61440 where p > c (anti-causal), 0 elsewhere;
            # raw scores are 256x so -61440*exp_scale = -30 -> exp ~ 1e-13
            maskU = consts.tile([128, 128], bf16, tag="maskU")
            nc.gpsimd.memset(maskU[:], -61440.0)
            nc.gpsimd.affine_select(
                out=maskU[:], in_=maskU[:], compare_op=mybir.AluOpType.is_gt,
                fill=0.0, base=0, pattern=[[-1, 128]], channel_multiplier=1,
            )
            identb = consts.tile([128, 128], bf16, tag="identb")
            nc.vector.tensor_copy(identb[:], ident[:])
            ones = consts.tile([1, 512], f32, tag="ones")
            nc.vector.memset(ones[:], 1.0)
            ones_t = consts.tile([128, 64], bf16, tag="ones_t")
            nc.vector.memset(ones_t[:], 1.0)

            # ---- weights / biases ----
            # fp8 W (16x) with ci-pair layout [128, pair, parity, 512]
            w8 = {}
            b_row = {}
            with tc.tile_pool(name="wtmp", bufs=3) as wtmp_pool:
                for name, wdram in (("q", Wq), ("k", Wk), ("v", Wv)):
                    t = persist.tile([128, NP, 2, 512], f8, tag=f"W8{name}",
                                     name=f"W8{name}")
                    for c in range(NCH):
                        wt = wtmp_pool.tile([128, 512], f32, tag="wtmp",
                                            name="wtmp")
                        nc.gpsimd.dma_start(
                            wt[:], wdram[128 * c:128 * (c + 1), :])
                        nc.vector.tensor_scalar_mul(
                            t[:, c // 2, c % 2, :], wt[:], 16.0)
                    w8[name] = t
                # bf16 16x Wv for the early-query exception V tiles
                wv16 = persist.tile([128, NCH, 512], bf16, tag="Wv16",
                                    name="Wv16")
                for c in range(NCH):
                    wt = wtmp_pool.tile([128, 512], f32, tag="wtmp",
                                        name="wtmp")
                    nc.gpsimd.dma_start(wt[:], Wv[128 * c:128 * (c + 1), :])
                    nc.vector.tensor_scalar_mul(wv16[:, c, :], wt[:], 16.0)
                # f32 Wo (f32r out-proj as before)
                wo_sb = persist.tile([128, NCH, 512], f32, tag="Wo", name="Wo")
                for c in range(NCH):
                    wt = wtmp_pool.tile([128, 512], f32, tag="wtmp",
                                        name="wtmp")
                    nc.gpsimd.dma_start(wt[:], Wo[128 * c:128 * (c + 1), :])
                    nc.vector.tensor_copy(r(wo_sb[:, c, :]), wt[:])
                for name, bdram in (("q", bq), ("k", bk), ("v", bv), ("o", bo)):
                    t = wtmp_pool.tile([1, 512], f32, tag=f"b{name}",
                                       name=f"b{name}", bufs=1)
                    nc.gpsimd.dma_start(t[:], bdram[None, :])
                    b_row[name] = t
                # per-partition bias columns for q/k (dout on partitions),
                # scaled 16x to match the fp8 weight scale
                bcol = {}
                for name in ("q", "k"):
                    row16 = wtmp_pool.tile([1, 512], f32, tag=f"r16{name}",
                                           name=f"r16{name}", bufs=2)
                    nc.vector.tensor_scalar_mul(row16[:], b_row[name][:], 16.0)
                    bc_t = consts.tile([128, NCH], f32, tag=f"bcol{name}",
                                       name=f"bcol{name}")
                    for c in range(NCH):
                        tp = ps512.tile([128, 512], f32, tag="ps512",
                                        name="ps512")
                        nc.tensor.transpose(
                            tp[:, 0:1], row16[0:1, 128 * c:128 * (c + 1)],
                            ident[0:1, 0:1])
                        nc.vector.tensor_copy(bc_t[:, c:c + 1], tp[:, 0:1])
                    bcol[name] = bc_t
                # broadcast bias tiles: v (head-interleaved, 16x) and o
                bvb = consts.tile([128, H, DH], f32, tag="bvb", name="bvb")
                bob = consts.tile([128, 512], f32, tag="bob", name="bob")
                for dst, row, scl in ((bvb, b_row["v"], 16.0),
                                      (bob, b_row["o"], 1.0)):
                    rowr = wtmp_pool.tile([1, 512], f32, tag="browr",
                                          name="browr", bufs=2)
                    nc.vector.tensor_scalar_mul(r(rowr[:]), row[:], scl)
                    tp = ps512.tile([128, 512], f32, tag="ps512", name="ps512")
                    nc.tensor.matmul(tp[:], r(ones[0:1, 0:128]), r(rowr[:]),
                                     start=True, stop=True)
                    if dst is bvb:
                        nc.vector.tensor_copy(
                            dst[:], tp[:].rearrange("p (h d) -> p h d", h=H))
                    else:
                        nc.vector.tensor_copy(dst[:], tp[:])

            # ---- persistent activations ----
            kt_sb = [persist.tile([128, L], f32, tag=f"KT{c}", name=f"KT{c}")
                     for c in range(NCH)]
            # fp8 V pairs: [128, parity, H, 65]; 65th col = 16.0 (denom)
            # padded to DH+4: fp8 ldweights rows must be 4B aligned
            v8_sb = [persist.tile([128, 2, H, DH + 4], f8, tag=f"V8{t}",
                        name=f"V8{t}") for t in range(NT // 2)]
            # bf16 16x V for tiles 0,1 (early-query exception)
            vbf_sb = [persist.tile([128, H, DH + 4], bf16, tag=f"Vb{t}",
                         name=f"Vb{t}") for t in range(2)]
            stage = [persist.tile([128, L], f32, tag=f"stage{c}",
                        name=f"stage{c}") for c in range(NCH)]

            with (
                tc.tile_pool(name="xin", bufs=XIN) as xin_pool,
                tc.tile_pool(name="qtg", bufs=2) as qtg_pool,
                tc.tile_pool(name="xt", bufs=XT) as xt_pool,
                tc.tile_pool(name="pexp", bufs=PEXP) as p_pool,
                tc.tile_pool(name="norm", bufs=1) as norm_pool,
                tc.tile_pool(name="osb", bufs=OSB) as o_pool,
            ):
                def emit_a_pieces(g):
                    qt_g = [qtg_pool.tile([128, 512], f32, tag=f"qtg{c}",
                                          name=f"qtg{c}") for c in range(NCH)]
                    pieces = []
                    state = {}
                    for tname_, xdram_ in (("k", xk), ("v", xv), ("q", xq)):
                        pieces.append(
                            lambda tname=tname_, xdram=xdram_:
                            state.__setitem__(
                                tname, emit_a_transpose(g, xdram,
                                                        tname == "v")))
                        pieces.append(
                            lambda tname=tname_: emit_a_proj(
                                g, tname, state[tname], qt_g))
                    return qt_g, pieces

                def emit_a_transpose(g, xdram, is_v):
                    xtiles = []
                    for j in range(4):
                        t0 = 4 * g + j
                        xt_in = xin_pool.tile([128, 512], f32, tag="xin",
                                              name="xin")
                        nc.sync.dma_start(
                            xt_in[:], xdram[128 * t0:128 * (t0 + 1), :])
                        # cast to bf16 on the Pool engine (fp8 transposes
                        # are illegal: walrus requires element step 2);
                        # the fp8 cast happens in the PSUM->SBUF copy.
                        xb = xt_pool.tile([128, 512], bf16, tag="xb",
                                          name="xb", bufs=6)
                        nc.gpsimd.tensor_copy(xb[:], xt_in[:])
                        xtiles.append(xb)
                        if is_v and g == 0 and j < 2:
                            xtiles.append(("bf", xb))
                    # bf16 transposes, channel-pairs packed into one PSUM
                    # bank; DVE copies PSUM->SBUF at 2x (bf16), Pool does
                    # the SBUF->SBUF fp8 cast (Pool cannot touch PSUM).
                    xt_c = []
                    bf_tiles = [x[1] for x in xtiles
                                if isinstance(x, tuple)]
                    b_tiles = [x for x in xtiles if not isinstance(x, tuple)]
                    for P in range(NP):
                        ps = ps512.tile([128, 512], f32, tag="ps512",
                                        name="ps512")
                        psb = ps[:].bitcast(bf16)
                        for cc in range(2):
                            c = 2 * P + cc
                            for j in range(4):
                                nc.tensor.transpose(
                                    psb[:, 512 * cc + 128 * j:
                                        512 * cc + 128 * (j + 1)],
                                    b_tiles[j][:, 128 * c:128 * (c + 1)],
                                    identb[:],
                                )
                        xtb = xt_pool.tile([128, 1024], bf16, tag="xtb",
                                           name="xtb", bufs=4)
                        nc.vector.tensor_copy(xtb[:], psb[:])
                        pr = xt_pool.tile([128, 2, 512], f8, tag="xt",
                                          name="xt", bufs=8)
                        nc.gpsimd.tensor_copy(
                            pr[:].rearrange("p a b -> p (a b)"), xtb[:])
                        xt_c.append(pr)
                    xt_bf = None
                    if bf_tiles:
                        xt_bf = xt_pool.tile([128, NCH, 256], bf16,
                                             tag="xtbf", name="xtbf", bufs=1)
                        for c in range(NCH):
                            ps = ps512.tile([128, 512], f32, tag="ps512",
                                            name="ps512")
                            psb = ps[:, 0:128].bitcast(bf16)
                            for j in range(2):
                                nc.tensor.transpose(
                                    psb[:, 128 * j:128 * (j + 1)],
                                    bf_tiles[j][:, 128 * c:128 * (c + 1)],
                                    identb[:],
                                )
                            nc.vector.tensor_copy(xt_bf[:, c, :], psb[:])
                    return (xt_c, xt_bf)

                def emit_a_proj(g, tname, xt_state, qt_g):
                    xt_c, xt_bf = xt_state
                    if tname in ("q", "k"):
                        for co in range(NCH):
                            pp = ps512.tile([128, 512], f32, tag="ps512",
                                            name="ps512")
                            for P in range(NP):
                                nc.tensor.matmul(
                                    pp[:],
                                    w8[tname][:, P, :,
                                              128 * co:128 * (co + 1)],
                                    xt_c[P][:],
                                    start=(P == 0), stop=(P == NP - 1),
                                    perf_mode=DR,
                                )
                            if tname == "q":
                                nc.vector.tensor_scalar_add(
                                    r(qt_g[co][:]), pp[:],
                                    bcol["q"][:, co:co + 1])
                            else:
                                nc.vector.tensor_scalar_add(
                                    r(kt_sb[co][:, 512 * g:512 * (g + 1)]),
                                    pp[:], bcol["k"][:, co:co + 1])
                    else:
                        for j in range(4):
                            t0 = 4 * g + j
                            pv = ps512.tile([128, 512], f32, tag="ps512",
                                            name="ps512")
                            for P in range(NP):
                                nc.tensor.matmul(
                                    pv[:],
                                    xt_c[P][:, :, 128 * j:128 * (j + 1)],
                                    w8["v"][:, P, :, :],
                                    start=(P == 0), stop=(P == NP - 1),
                                    perf_mode=DR,
                                )
                            dst = v8_sb[t0 // 2]
                            nc.vector.tensor_add(
                                dst[:, t0 % 2, :, 0:DH],
                                pv[:].rearrange("p (h d) -> p h d", h=H),
                                bvb[:],
                            )
                            nc.gpsimd.memset(
                                dst[:, t0 % 2, :, DH:DH + 1], 16.0)
                            nc.gpsimd.memset(
                                dst[:, t0 % 2, :, DH + 1:DH + 4], 0.0)
                        if xt_bf is not None:
                            # bf16 V for tiles 0,1
                            for j in range(2):
                                pv = ps512.tile([128, 512], f32, tag="ps512",
                                                name="ps512")
                                for ci in range(NCH):
                                    nc.tensor.matmul(
                                        pv[:],
                                        xt_bf[:, ci, 128 * j:128 * (j + 1)],
                                        wv16[:, ci, :],
                                        start=(ci == 0), stop=(ci == NCH - 1),
                                    )
                                nc.vector.tensor_add(
                                    vbf_sb[j][:, :, 0:DH],
                                    pv[:].rearrange("p (h d) -> p h d", h=H),
                                    bvb[:],
                                )
                                nc.gpsimd.memset(
                                    vbf_sb[j][:, :, DH:DH + 1], 16.0)
                                nc.gpsimd.memset(
                                    vbf_sb[j][:, :, DH + 1:DH + 4], 0.0)

                def emit_b_qt(qt, qt_g, weave=()):
                    weave = list(weave)
                    kmax = 4 * qt + 4
                    npair = kmax // 2
                    stg = norm_pool.tile([128, 1536], f32, tag="stg",
                                         name="stg")
                    # the reciprocal reads the full tile (HW forbids strided
                    # partition APs); init the never-written rows on Pool
                    nc.gpsimd.memset(stg[:], 1.0)
                    for hp in range(H // 2):
                        ch = hp
                        kth = kt_sb[ch]
                        qth = qt_g[ch]
                        po = [ops_pool.tile([68, 512], f32, tag="ops",
                                            name="ops") for _ in range(2)]

                        def emit_av(p, pt, is_bf, stop):
                            m0 = 2 * p - 4 * qt
                            if is_bf:
                                # exception (qt=0 pair 0): tiles 0,1 bf16
                                for cc in range(2):
                                    jv = 128 * cc
                                    for k in range(2):
                                        nc.tensor.matmul(
                                            po[k][:, jv:512],
                                            vbf_sb[cc][:, 2 * hp + k, :],
                                            pt[:, cc, k, jv:512],
                                            start=(cc == 0), stop=False,
                                            skip_group_check=True,
                                        )
                                return
                            jv0 = 0 if m0 < 1 else 128 * m0
                            for k in range(2):
                                nc.tensor.matmul(
                                    po[k][:, jv0:512],
                                    v8_sb[p][:, :, 2 * hp + k, :],
                                    pt[:, :, k, jv0:512],
                                    start=(p == 0 and qt > 0), stop=stop,
                                    perf_mode=DR,
                                    skip_group_check=True,
                                )

                        pending = None
                        dbg_pt0_src = []
                        for p in range(npair):
                            is_bf = (qt == 0 and p == 0)
                            if is_bf:
                                pt = p_pool.tile([128, 2, 2, 512], bf16,
                                                 tag="pexpb", name="pexpb",
                                                 bufs=2)
                                if DEBUG and hp == 0:
                                    dbg_pt0_src.append(pt)
                                if DEBUG:
                                    nc.gpsimd.memset(pt[:, 1, :, 0:128], 0.0)
                            else:
                                pt = p_pool.tile([128, 2, 2, 512], f8,
                                                 tag="pexp", name="pexp")
                                m0 = 2 * p - 4 * qt
                                if m0 == 0:
                                    nc.gpsimd.memset(pt[:, 1, :, 0:128], 0.0)
                                elif m0 == 2:
                                    nc.gpsimd.memset(pt[:, 1, :, 256:384], 0.0)
                            for cc in range(2):
                                c = 2 * p + cc
                                m = c - 4 * qt
                                js0 = 0 if m < 1 else (128 * m if m < 3
                                                       else 256)
                                jv0 = 0 if m < 1 else 128 * m
                                ps = sps2_pool.tile([128, 2, 512], f32,
                                                    tag="sps2", name="sps2")
                                for k in range(2):
                                    prow = 64 * k
                                    # stop=True even when the mask follows:
                                    # stop is a HW no-op, and CoreSim's group
                                    # accounting ignores bf16 (ldweights-
                                    # paired) matmul flags, so the f32r
                                    # score must close its own group.
                                    nc.tensor.matmul(
                                        ps[:, k, js0:512],
                                        r(kth[prow:prow + DH,
                                              128 * c:128 * (c + 1)]),
                                        r(qth[prow:prow + DH, js0:512]),
                                        start=True, stop=True,
                                    )
                                if m >= 0:
                                    # -61440 above the diagonal (PE-side)
                                    for k in range(2):
                                        nc.tensor.matmul(
                                            ps[:, k, 128 * m:128 * (m + 1)],
                                            identb[:], maskU[:],
                                            start=False, stop=True,
                                            skip_group_check=True,
                                        )
                                nc.scalar.activation(
                                    pt[:, cc, :, jv0:512], ps[:, :, jv0:512],
                                    mybir.ActivationFunctionType.Exp,
                                    scale=0.125 / 256.0,
                                )
                            if pending is not None:
                                emit_av(*pending, stop=False)
                            pending = (p, pt, is_bf)
                        emit_av(*pending, stop=True)
                        if DEBUG and qt == 0 and hp == 0:
                            po_sb = norm_pool.tile([65, 512], f32,
                                                   tag="dbgpo", name="dbgpo")
                            nc.vector.tensor_copy(po_sb[:], po[0][0:65, :])
                            nc.sync.dma_start(dbg["po0"][:], po_sb[:])
                            nc.sync.dma_start(dbg["kt0"][:],
                                              kt_sb[0][:, 0:512])
                            nc.sync.dma_start(dbg["qt0"][:], qt_g[0][:])
                            nc.sync.dma_start(dbg["vbf0"][:], vbf_sb[0][:])
                            nc.sync.dma_start(dbg["v80"][:], v8_sb[0][:])
                            nc.sync.dma_start(dbg["pt0"][:], dbg_pt0_src[0][:])
                        for k in range(2):
                            h = 2 * hp + k
                            prow = 64 * k
                            nc.vector.tensor_copy(
                                r(stage[ch][prow:prow + DH,
                                            512 * qt:512 * (qt + 1)]),
                                po[k][0:DH, :])
                            nc.vector.tensor_copy(
                                stg[32 * (h % 3):32 * (h % 3) + 1,
                                    512 * (h // 3):512 * (h // 3) + 512],
                                po[k][DH:DH + 1, :])
                        if weave and hp >= 1:
                            weave.pop(0)()
                            if weave:
                                weave.pop(0)()
                    rstg = norm_pool.tile([128, 1536], f32, tag="rstg",
                                          name="rstg")
                    nc.vector.reciprocal_approx_fast(out=rstg[:], in_=stg[:])
                    rbf = norm_pool.tile([128, 1536], bf16, tag="rbf",
                                         name="rbf")
                    nc.vector.tensor_copy(rbf[:], rstg[:])
                    for ch in range(NCH):
                        bcp = ps512.tile([128, 512], f32, tag="ps512",
                                         name="ps512")
                        for sub in range(2):
                            hh = 2 * ch + sub
                            pp0 = 32 * (hh % 3)
                            fo = 512 * (hh // 3)
                            nc.tensor.matmul(
                                bcp[64 * sub:64 * sub + 64, :],
                                ones_t[pp0:pp0 + 1, 0:64],
                                rbf[pp0:pp0 + 1, fo:fo + 512],
                                start=True, stop=True,
                            )
                        nc.vector.tensor_mul(
                            r(stage[ch][:, 512 * qt:512 * (qt + 1)]),
                            stage[ch][:, 512 * qt:512 * (qt + 1)],
                            bcp[:],
                        )
                    for i in range(4 * qt, 4 * qt + 4):
                        pout = ps512.tile([128, 512], f32, tag="ps512",
                                          name="ps512")
                        for ch in range(NCH):
                            nc.tensor.matmul(
                                pout[:],
                                r(stage[ch][:, 128 * i:128 * (i + 1)]),
                                r(wo_sb[:, ch, :]),
                                start=(ch == 0), stop=(ch == NCH - 1),
                            )
                        ot = o_pool.tile([128, 512], f32, tag="osb",
                                         name="osb")
                        nc.vector.tensor_add(ot[:], pout[:], bob[:])
                        nc.sync.dma_start(out[128 * i:128 * (i + 1), :], ot[:])
                    for w in weave:
                        w()

                def emit_body():
                    qt_g, pieces = emit_a_pieces(0)
                    for p in pieces:
                        p()
                    for g in range(NQ):
                        if g + 1 < NQ:
                            qt_next, weave = emit_a_pieces(g + 1)
                        else:
                            qt_next, weave = None, ()
                        emit_b_qt(g, qt_g, weave)
                        qt_g = qt_next

                def apply_extra_deps():
                    if not extra_deps:
                        return
                    from concourse.tile_rust import add_dep_helper
                    by_name = {i.name: i
                               for i in _walk_instructions(nc.cur_f)}
                    for fr, to in extra_deps:
                        fi, ti = by_name.get(fr), by_name.get(to)
                        if fi is not None and ti is not None:
                            add_dep_helper(fi, ti, sync=True,
                                           reason="audit fixpoint WAR")

                if repeat > 1:
                    with tc.For_i(0, repeat, 1, hint_engines=(
                            mybir.EngineType.PE,
                            mybir.EngineType.DVE,
                            mybir.EngineType.Activation,
                            mybir.EngineType.SP,
                            mybir.EngineType.Pool)):
                        emit_body()
                        _add_reuse_guards(tc, nc)
                        apply_extra_deps()
                else:
                    emit_body()
                    _add_reuse_guards(tc, nc)
                    apply_extra_deps()

    nc.compile()
    return nc


def get_nc(repeat=1, cfg=None):
    key = f"nc{repeat}-{sorted((cfg or {}).items())}"
    if key not in _cached:
        deps = set()
        nc = None
        for _ in range(4):
            nc = _build(repeat, cfg, extra_deps=sorted(deps))
            missing = _audit_war(nc)
            new = missing - deps
            if not new:
                break
            deps |= new
        _cached[key] = nc
    return _cached[key]


def run(in_maps, trace=False, repeat=1, cfg=None, **kw):
    from concourse.bass_utils import run_bass_kernel_spmd

    nc = get_nc(repeat, cfg)
    return run_bass_kernel_spmd(nc, in_maps, list(range(B)), trace=trace, **kw)


def kernel(query, key, value, Wq, bq, Wk, bk, Wv, bv, Wo, bo):
    shared = {
        "Wq": np.ascontiguousarray(Wq, np.float32),
        "Wk": np.ascontiguousarray(Wk, np.float32),
        "Wv": np.ascontiguousarray(Wv, np.float32),
        "Wo": np.ascontiguousarray(Wo, np.float32),
        "bq": np.ascontiguousarray(bq, np.float32),
        "bk": np.ascontiguousarray(bk, np.float32),
        "bv": np.ascontiguousarray(bv, np.float32),
        "bo": np.ascontiguousarray(bo, np.float32),
    }
    in_maps = []
    for i in range(B):
        m = dict(shared)
        m["query"] = np.ascontiguousarray(query[i], np.float32)
        m["key"] = np.ascontiguousarray(key[i], np.float32)
        m["value"] = np.ascontiguousarray(value[i], np.float32)
        in_maps.append(m)
    res = run(in_maps)
    return np.stack([res.results[i]["out"] for i in range(B)], axis=0)


# revision 35
# speedup vs baseline: 1.1258x; 1.1258x over previous
"""v15: v3 weave + software-pipelined attnV + PE-side causal mask
+ bf16 transpose path.

- attn@V for key tile c is emitted one step behind the score/exp pair of
  tile c+1, so the in-order PE queue never stalls waiting for an exp it
  could have overlapped with the next score matmuls.
- the causal mask on diagonal 128-blocks is applied by accumulating a
  constant -240 upper-triangle into the score PSUM (one extra 128-wide
  matmul, start=False) BEFORE exp, instead of a post-exp DVE tensor_mul.
- inputs are cast to bf16 on the idle Pool engine before the PE
  transposes (1 c/row instead of 2 for f32); Q/K/V projections then run
  bf16. Transpose output lands in a bf16 bitcast view of the f32 PSUM
  tile (no extra banks); scores/out-proj stay f32r as before.
Measured: ~154us test.py slope (v13: 206, v9: 296.3, v3: 314.0),
rel err 4.4e-3.

(v16, an fp8-DoubleRow rework, was built, CoreSim-validated and HW-run
in this session: correct at rel 1.15e-2 but measured 379-393us vs
v15's 324-368us slope here -- DoubleRow's modeled 0.5 c/row does not
materialize on this hardware and fp8 Ldweights/Pool-cast overheads
dominate. Reverted to v15. v16 kept at /tmp/kernel_v16_backup.py.)
"""

import numpy as np

B = 8
L = 2048
D = 512
H = 8
DH = 64
NT = L // 128
NCH = D // 128
NQ = L // 512

_cached = {}


def _build(repeat=1, cfg=None):
    cfg = dict(cfg or {})
    PS512 = cfg.get("ps512", 2)
    SPS2 = cfg.get("sps2", 2)
    OPS = cfg.get("ops", 2)
    PEXP = cfg.get("pexp", 6)
    OSB = cfg.get("osb", 3)
    XIN = cfg.get("xin", 8)
    XT = cfg.get("xt", 10)
    import concourse.tile as tile
    from concourse import mybir, bacc
    from concourse.masks import make_identity

    f32 = mybir.dt.float32
    bf16 = mybir.dt.bfloat16
    f32r = mybir.dt.float32r

    nc = bacc.Bacc("TRN2", target_bir_lowering=False, debug=False)

    xq = nc.dram_tensor("query", [L, D], f32, kind="ExternalInput").ap()
    xk = nc.dram_tensor("key", [L, D], f32, kind="ExternalInput").ap()
    xv = nc.dram_tensor("value", [L, D], f32, kind="ExternalInput").ap()
    Wq = nc.dram_tensor("Wq", [D, D], f32, kind="ExternalInput").ap()
    Wk = nc.dram_tensor("Wk", [D, D], f32, kind="ExternalInput").ap()
    Wv = nc.dram_tensor("Wv", [D, D], f32, kind="ExternalInput").ap()
    Wo = nc.dram_tensor("Wo", [D, D], f32, kind="ExternalInput").ap()
    bq = nc.dram_tensor("bq", [D], f32, kind="ExternalInput").ap()
    bk = nc.dram_tensor("bk", [D], f32, kind="ExternalInput").ap()
    bv = nc.dram_tensor("bv", [D], f32, kind="ExternalInput").ap()
    bo = nc.dram_tensor("bo", [D], f32, kind="ExternalInput").ap()
    out = nc.dram_tensor("out", [L, D], f32, kind="ExternalOutput").ap()

    def r(ap):
        return ap.bitcast(f32r)

    with tile.TileContext(nc) as tc:
        with (
            tc.tile_pool(name="persist", bufs=1) as persist,
            tc.tile_pool(name="consts", bufs=1) as consts,
            tc.tile_pool(name="ps512", bufs=PS512, space="PSUM") as ps512,
            tc.tile_pool(name="sps2", bufs=SPS2, space="PSUM") as sps2_pool,
            tc.tile_pool(name="ops", bufs=OPS, space="PSUM") as ops_pool,
        ):
            # ---- constants ----
            ident = consts.tile([128, 128], f32, tag="ident")
            make_identity(nc, ident[:])
            # maskU[p, c] = -240 where p > c (anti-causal), 0 elsewhere;
            # added to the diagonal score block pre-exp so exp gives ~1e-13
            maskU = consts.tile([128, 128], bf16, tag="maskU")
            nc.gpsimd.memset(maskU[:], -240.0)
            nc.gpsimd.affine_select(
                out=maskU[:], in_=maskU[:], compare_op=mybir.AluOpType.is_gt,
                fill=0.0, base=0, pattern=[[-1, 128]], channel_multiplier=1,
            )
            identb = consts.tile([128, 128], bf16, tag="identb")
            nc.vector.tensor_copy(identb[:], ident[:])
            ones = consts.tile([1, 512], f32, tag="ones")
            nc.vector.memset(ones[:], 1.0)
            ones_t = consts.tile([128, 64], bf16, tag="ones_t")
            nc.vector.memset(ones_t[:], 1.0)

            # ---- weights / biases ----
            w_sb = {}
            b_row = {}
            with tc.tile_pool(name="wtmp", bufs=3) as wtmp_pool:
                for name, wdram in (("q", Wq), ("k", Wk), ("v", Wv), ("o", Wo)):
                    dt = f32 if name == "o" else bf16
                    t = persist.tile([128, NCH, 512], dt, tag=f"W{name}",
                                     name=f"W{name}")
                    for c in range(NCH):
                        wt = wtmp_pool.tile([128, 512], f32, tag="wtmp",
                                            name="wtmp")
                        nc.gpsimd.dma_start(
                            wt[:], wdram[128 * c:128 * (c + 1), :])
                        if name == "o":
                            nc.vector.tensor_copy(r(t[:, c, :]), wt[:])
                        else:
                            nc.vector.tensor_copy(t[:, c, :], wt[:])
                    w_sb[name] = t
                for name, bdram in (("q", bq), ("k", bk), ("v", bv), ("o", bo)):
                    t = wtmp_pool.tile([1, 512], f32, tag=f"b{name}",
                                       name=f"b{name}", bufs=1)
                    nc.gpsimd.dma_start(t[:], bdram[None, :])
                    b_row[name] = t
                # per-partition bias columns for q/k (dout on partitions)
                bcol = {}
                for name in ("q", "k"):
                    bc_t = consts.tile([128, NCH], f32, tag=f"bcol{name}",
                                       name=f"bcol{name}")
                    for c in range(NCH):
                        tp = ps512.tile([128, 512], f32, tag="ps512", name="ps512")
                        nc.tensor.transpose(
                            tp[:, 0:1], b_row[name][0:1, 128 * c:128 * (c + 1)],
                            ident[0:1, 0:1])
                        nc.vector.tensor_copy(bc_t[:, c:c + 1], tp[:, 0:1])
                    bcol[name] = bc_t
                # broadcast bias tiles for v (head-interleaved) and o (natural)
                bvb = consts.tile([128, H, DH], f32, tag="bvb", name="bvb")
                bob = consts.tile([128, 512], f32, tag="bob", name="bob")
                for dst, row in ((bvb, b_row["v"]), (bob, b_row["o"])):
                    rowr = wtmp_pool.tile([1, 512], f32, tag="browr",
                                          name="browr", bufs=2)
                    nc.vector.tensor_copy(r(rowr[:]), row[:])
                    tp = ps512.tile([128, 512], f32, tag="ps512", name="ps512")
                    nc.tensor.matmul(tp[:], r(ones[0:1, 0:128]), r(rowr[:]),
                                     start=True, stop=True)
                    if dst is bvb:
                        nc.vector.tensor_copy(
                            dst[:], tp[:].rearrange("p (h d) -> p h d", h=H))
                    else:
                        nc.vector.tensor_copy(dst[:], tp[:])

            # ---- persistent activations ----
            kt_sb = [persist.tile([128, L], f32, tag=f"KT{c}", name=f"KT{c}")
                     for c in range(NCH)]
            v_sb = [persist.tile([128, H, DH + 1], bf16, tag=f"V{t}",
                        name=f"V{t}") for t in range(NT)]
            stage = [persist.tile([128, L], f32, tag=f"stage{c}", name=f"stage{c}")
                     for c in range(NCH)]

            with (
                tc.tile_pool(name="xin", bufs=XIN) as xin_pool,
                tc.tile_pool(name="qtg", bufs=2) as qtg_pool,
                tc.tile_pool(name="xt", bufs=XT) as xt_pool,
                tc.tile_pool(name="pexp", bufs=PEXP) as p_pool,
                tc.tile_pool(name="norm", bufs=1) as norm_pool,
                tc.tile_pool(name="osb", bufs=OSB) as o_pool,
            ):
                def emit_a_pieces(g):
                    qt_g = [qtg_pool.tile([128, 512], f32, tag=f"qtg{c}",
                                          name=f"qtg{c}") for c in range(NCH)]
                    pieces = []
                    state = {}
                    for tname_, xdram_ in (("k", xk), ("v", xv), ("q", xq)):
                        pieces.append(
                            lambda tname=tname_, xdram=xdram_:
                            state.__setitem__(
                                tname, emit_a_transpose(g, xdram)))
                        pieces.append(
                            lambda tname=tname_: emit_a_proj(
                                g, tname, state[tname], qt_g))
                    return qt_g, pieces

                def emit_a_transpose(g, xdram):
                    if True:
                        xtiles = []
                        for j in range(4):
                            t0 = 4 * g + j
                            xt_in = xin_pool.tile([128, 512], f32, tag="xin",
                                                  name="xin")
                            nc.sync.dma_start(
                                xt_in[:], xdram[128 * t0:128 * (t0 + 1), :])
                            # cast to bf16 on the idle Pool engine so the
                            # transposes run at 1 c/row instead of 2
                            xb = xt_pool.tile([128, 512], bf16, tag="xb",
                                              name="xb", bufs=6)
                            nc.gpsimd.tensor_copy(xb[:], xt_in[:])
                            xtiles.append(xb)
                        xt_c = []
                        for c in range(NCH):
                            ps = ps512.tile([128, 512], f32, tag="ps512",
                                            name="ps512")
                            psb = ps[:, 0:256].bitcast(bf16)
                            for j in range(4):
                                nc.tensor.transpose(
                                    psb[:, 128 * j:128 * (j + 1)],
                                    xtiles[j][:, 128 * c:128 * (c + 1)],
                                    identb[:],
                                )
                            sb = xt_pool.tile([128, 512], bf16, tag="xt",
                                              name="xt")
                            nc.vector.tensor_copy(sb[:], psb)
                            xt_c.append(sb)
                        return xt_c

                def emit_a_proj(g, tname, xt_c, qt_g):
                    if True:
                        if tname in ("q", "k"):
                            for co in range(NCH):
                                pp = ps512.tile([128, 512], f32, tag="ps512",
                                                name="ps512")
                                for ci in range(NCH):
                                    nc.tensor.matmul(
                                        pp[:],
                                        w_sb[tname][
                                            :, ci, 128 * co:128 * (co + 1)],
                                        xt_c[ci][:],
                                        start=(ci == 0), stop=(ci == NCH - 1),
                                    )
                                if tname == "q":
                                    nc.vector.tensor_scalar_add(
                                        r(qt_g[co][:]), pp[:],
                                        bcol["q"][:, co:co + 1])
                                else:
                                    nc.vector.tensor_scalar_add(
                                        r(kt_sb[co][:, 512 * g:512 * (g + 1)]),
                                        pp[:], bcol["k"][:, co:co + 1])
                        else:
                            for j in range(4):
                                t0 = 4 * g + j
                                pv = ps512.tile([128, 512], f32, tag="ps512",
                                                name="ps512")
                                for ci in range(NCH):
                                    nc.tensor.matmul(
                                        pv[:],
                                        xt_c[ci][:, 128 * j:128 * (j + 1)],
                                        w_sb["v"][:, ci, :],
                                        start=(ci == 0), stop=(ci == NCH - 1),
                                    )
                                nc.vector.tensor_add(
                                    v_sb[t0][:, :, 0:DH],
                                    pv[:].rearrange("p (h d) -> p h d", h=H),
                                    bvb[:],
                                )
                                nc.gpsimd.memset(v_sb[t0][:, :, DH:DH + 1], 1.0)

                def emit_b_qt(qt, qt_g, weave=()):
                    weave = list(weave)
                    kmax = 4 * qt + 4
                    stg = norm_pool.tile([128, 1536], f32, tag="stg", name="stg")
                    for hp in range(H // 2):
                        ch = hp
                        kth = kt_sb[ch]
                        qth = qt_g[ch]
                        po = [ops_pool.tile([65, 512], f32, tag="ops",
                                            name="ops") for _ in range(2)]

                        def emit_av(c, pt):
                            m = c - 4 * qt
                            jv0 = 0 if m < 1 else 128 * m
                            for k in range(2):
                                nc.tensor.matmul(
                                    po[k][:, jv0:512],
                                    v_sb[c][:, 2 * hp + k, :],
                                    pt[:, 512 * k + jv0:512 * (k + 1)],
                                    start=(c == 0), stop=(c == kmax - 1),
                                )

                        pending = None
                        for c in range(kmax):
                            m = c - 4 * qt
                            js0 = 0 if m < 1 else (128 * m if m < 3 else 256)
                            jv0 = 0 if m < 1 else 128 * m
                            ps = sps2_pool.tile([128, 1024], f32, tag="sps2",
                                                name="sps2")
                            pt = p_pool.tile([128, 1024], bf16, tag="pexp",
                                             name="pexp")
                            for k in range(2):
                                prow = 64 * k
                                nc.tensor.matmul(
                                    ps[:, 512 * k + js0:512 * (k + 1)],
                                    r(kth[prow:prow + DH,
                                          128 * c:128 * (c + 1)]),
                                    r(qth[prow:prow + DH, js0:512]),
                                    start=True, stop=(m < 0),
                                )
                            if m < 0:
                                nc.scalar.activation(
                                    pt[:], ps[:],
                                    mybir.ActivationFunctionType.Exp,
                                    scale=0.125,
                                )
                            else:
                                # accumulate -240 above the diagonal of the
                                # 128-wide diag block (PE, no DVE hop)
                                for k in range(2):
                                    nc.tensor.matmul(
                                        ps[:, 512 * k + 128 * m:
                                           512 * k + 128 * (m + 1)],
                                        identb[:], maskU[:],
                                        start=False, stop=True,
                                        skip_group_check=True,
                                    )
                                for k in range(2):
                                    nc.scalar.activation(
                                        pt[:, 512 * k + jv0:512 * (k + 1)],
                                        ps[:, 512 * k + jv0:512 * (k + 1)],
                                        mybir.ActivationFunctionType.Exp,
                                        scale=0.125,
                                    )
                            if pending is not None:
                                emit_av(*pending)
                            pending = (c, pt)
                        emit_av(*pending)
                        for k in range(2):
                            h = 2 * hp + k
                            prow = 64 * k
                            nc.vector.tensor_copy(
                                r(stage[ch][prow:prow + DH,
                                            512 * qt:512 * (qt + 1)]),
                                po[k][0:DH, :])
                            nc.vector.tensor_copy(
                                stg[32 * (h % 3):32 * (h % 3) + 1,
                                    512 * (h // 3):512 * (h // 3) + 512],
                                po[k][DH:DH + 1, :])
                        if weave and hp >= 1:
                            weave.pop(0)()
                            if weave:
                                weave.pop(0)()
                    rstg = norm_pool.tile([128, 1536], f32, tag="rstg",
                                          name="rstg")
                    nc.vector.reciprocal_approx_fast(out=rstg[:], in_=stg[:])
                    rbf = norm_pool.tile([128, 1536], bf16, tag="rbf", name="rbf")
                    nc.vector.tensor_copy(rbf[:], rstg[:])
                    for ch in range(NCH):
                        bcp = ps512.tile([128, 512], f32, tag="ps512",
                                         name="ps512")
                        for sub in range(2):
                            hh = 2 * ch + sub
                            pp0 = 32 * (hh % 3)
                            fo = 512 * (hh // 3)
                            nc.tensor.matmul(
                                bcp[64 * sub:64 * sub + 64, :],
                                ones_t[pp0:pp0 + 1, 0:64],
                                rbf[pp0:pp0 + 1, fo:fo + 512],
                                start=True, stop=True,
                            )
                        nc.vector.tensor_mul(
                            r(stage[ch][:, 512 * qt:512 * (qt + 1)]),
                            stage[ch][:, 512 * qt:512 * (qt + 1)],
                            bcp[:],
                        )
                    for i in range(4 * qt, 4 * qt + 4):
                        pout = ps512.tile([128, 512], f32, tag="ps512",
                                          name="ps512")
                        for ch in range(NCH):
                            nc.tensor.matmul(
                                pout[:],
                                r(stage[ch][:, 128 * i:128 * (i + 1)]),
                                r(w_sb["o"][:, ch, :]),
                                start=(ch == 0), stop=(ch == NCH - 1),
                            )
                        ot = o_pool.tile([128, 512], f32, tag="osb", name="osb")
                        nc.vector.tensor_add(ot[:], pout[:], bob[:])
                        nc.sync.dma_start(out[128 * i:128 * (i + 1), :], ot[:])
                    for w in weave:
                        w()

                def emit_body():
                    qt_g, pieces = emit_a_pieces(0)
                    for p in pieces:
                        p()
                    for g in range(NQ):
                        if g + 1 < NQ:
                            qt_next, weave = emit_a_pieces(g + 1)
                        else:
                            qt_next, weave = None, ()
                        emit_b_qt(g, qt_g, weave)
                        qt_g = qt_next

                if repeat > 1:
                    with tc.For_i(0, repeat, 1, hint_engines=(
                            mybir.EngineType.PE,
                            mybir.EngineType.DVE,
                            mybir.EngineType.Activation,
                            mybir.EngineType.SP,
                            mybir.EngineType.Pool)):
                        emit_body()
                else:
                    emit_body()

    nc.compile()
    return nc


def get_nc(repeat=1, cfg=None):
    key = f"nc{repeat}-{sorted((cfg or {}).items())}"
    if key not in _cached:
        _cached[key] = _build(repeat, cfg)
    return _cached[key]


def run(in_maps, trace=False, repeat=1, cfg=None, **kw):
    from concourse.bass_utils import run_bass_kernel_spmd

    nc = get_nc(repeat, cfg)
    return run_bass_kernel_spmd(nc, in_maps, list(range(B)), trace=trace, **kw)


def kernel(query, key, value, Wq, bq, Wk, bk, Wv, bv, Wo, bo):
    shared = {
        "Wq": np.ascontiguousarray(Wq, np.float32),
        "Wk": np.ascontiguousarray(Wk, np.float32),
        "Wv": np.ascontiguousarray(Wv, np.float32),
        "Wo": np.ascontiguousarray(Wo, np.float32),
        "bq": np.ascontiguousarray(bq, np.float32),
        "bk": np.ascontiguousarray(bk, np.float32),
        "bv": np.ascontiguousarray(bv, np.float32),
        "bo": np.ascontiguousarray(bo, np.float32),
    }
    in_maps = []
    for i in range(B):
        m = dict(shared)
        m["query"] = np.ascontiguousarray(query[i], np.float32)
        m["key"] = np.ascontiguousarray(key[i], np.float32)
        m["value"] = np.ascontiguousarray(value[i], np.float32)
        in_maps.append(m)
    res = run(in_maps)
    return np.stack([res.results[i]["out"] for i in range(B)], axis=0)


# revision 36
# speedup vs baseline: 1.1979x; 1.0641x over previous
"""v15: v3 weave + software-pipelined attnV + PE-side causal mask
+ bf16 transpose path.

- attn@V for key tile c is emitted one step behind the score/exp pair of
  tile c+1, so the in-order PE queue never stalls waiting for an exp it
  could have overlapped with the next score matmuls.
- the causal mask on diagonal 128-blocks is applied by accumulating a
  constant -240 upper-triangle into the score PSUM (one extra 128-wide
  matmul, start=False) BEFORE exp, instead of a post-exp DVE tensor_mul.
- inputs are cast to bf16 on the idle Pool engine before the PE
  transposes (1 c/row instead of 2 for f32); Q/K/V projections then run
  bf16. Transpose output lands in a bf16 bitcast view of the f32 PSUM
  tile (no extra banks); scores/out-proj stay f32r as before.
Measured: ~154us test.py slope (v13: 206, v9: 296.3, v3: 314.0),
rel err 4.4e-3.

(v16, an fp8-DoubleRow rework, was built, CoreSim-validated and HW-run
in this session: correct at rel 1.15e-2 but measured 379-393us vs
v15's 324-368us slope here -- DoubleRow's modeled 0.5 c/row does not
materialize on this hardware and fp8 Ldweights/Pool-cast overheads
dominate. Reverted to v15. v16 kept at /tmp/kernel_v16_backup.py.)
"""

import numpy as np

B = 8
L = 2048
D = 512
H = 8
DH = 64
NT = L // 128
NCH = D // 128
NQ = L // 512

_cached = {}


def _build(repeat=1, cfg=None):
    cfg = dict(cfg or {})
    PS512 = cfg.get("ps512", 2)
    SPS2 = cfg.get("sps2", 2)
    OPS = cfg.get("ops", 2)
    PEXP = cfg.get("pexp", 6)
    OSB = cfg.get("osb", 3)
    XIN = cfg.get("xin", 8)
    XT = cfg.get("xt", 10)
    import concourse.tile as tile
    from concourse import mybir, bacc
    from concourse.masks import make_identity

    f32 = mybir.dt.float32
    bf16 = mybir.dt.bfloat16
    f32r = mybir.dt.float32r

    nc = bacc.Bacc("TRN2", target_bir_lowering=False, debug=False)

    xq = nc.dram_tensor("query", [L, D], f32, kind="ExternalInput").ap()
    xk = nc.dram_tensor("key", [L, D], f32, kind="ExternalInput").ap()
    xv = nc.dram_tensor("value", [L, D], f32, kind="ExternalInput").ap()
    Wq = nc.dram_tensor("Wq", [D, D], f32, kind="ExternalInput").ap()
    Wk = nc.dram_tensor("Wk", [D, D], f32, kind="ExternalInput").ap()
    Wv = nc.dram_tensor("Wv", [D, D], f32, kind="ExternalInput").ap()
    Wo = nc.dram_tensor("Wo", [D, D], f32, kind="ExternalInput").ap()
    bq = nc.dram_tensor("bq", [D], f32, kind="ExternalInput").ap()
    bk = nc.dram_tensor("bk", [D], f32, kind="ExternalInput").ap()
    bv = nc.dram_tensor("bv", [D], f32, kind="ExternalInput").ap()
    bo = nc.dram_tensor("bo", [D], f32, kind="ExternalInput").ap()
    out = nc.dram_tensor("out", [L, D], f32, kind="ExternalOutput").ap()

    def r(ap):
        return ap.bitcast(f32r)

    with tile.TileContext(nc) as tc:
        with (
            tc.tile_pool(name="persist", bufs=1) as persist,
            tc.tile_pool(name="consts", bufs=1) as consts,
            tc.tile_pool(name="ps512", bufs=PS512, space="PSUM") as ps512,
            tc.tile_pool(name="sps2", bufs=SPS2, space="PSUM") as sps2_pool,
            tc.tile_pool(name="ops", bufs=OPS, space="PSUM") as ops_pool,
        ):
            # ---- constants ----
            ident = consts.tile([128, 128], f32, tag="ident")
            make_identity(nc, ident[:])
            # maskU[p, c] = -240 where p > c (anti-causal), 0 elsewhere;
            # added to the diagonal score block pre-exp so exp gives ~1e-13
            maskU = consts.tile([128, 128], bf16, tag="maskU")
            nc.gpsimd.memset(maskU[:], -240.0)
            nc.gpsimd.affine_select(
                out=maskU[:], in_=maskU[:], compare_op=mybir.AluOpType.is_gt,
                fill=0.0, base=0, pattern=[[-1, 128]], channel_multiplier=1,
            )
            identb = consts.tile([128, 128], bf16, tag="identb")
            nc.vector.tensor_copy(identb[:], ident[:])
            ones = consts.tile([1, 512], f32, tag="ones")
            nc.vector.memset(ones[:], 1.0)
            ones_t = consts.tile([128, 64], bf16, tag="ones_t")
            nc.vector.memset(ones_t[:], 1.0)

            # ---- weights / biases ----
            w_sb = {}
            b_row = {}
            with tc.tile_pool(name="wtmp", bufs=3) as wtmp_pool:
                for name, wdram in (("q", Wq), ("k", Wk), ("v", Wv), ("o", Wo)):
                    dt = f32 if name == "o" else bf16
                    t = persist.tile([128, NCH, 512], dt, tag=f"W{name}",
                                     name=f"W{name}")
                    for c in range(NCH):
                        wt = wtmp_pool.tile([128, 512], f32, tag="wtmp",
                                            name="wtmp")
                        nc.gpsimd.dma_start(
                            wt[:], wdram[128 * c:128 * (c + 1), :])
                        if name == "o":
                            nc.vector.tensor_copy(r(t[:, c, :]), wt[:])
                        else:
                            nc.vector.tensor_copy(t[:, c, :], wt[:])
                    w_sb[name] = t
                for name, bdram in (("q", bq), ("k", bk), ("v", bv), ("o", bo)):
                    t = wtmp_pool.tile([1, 512], f32, tag=f"b{name}",
                                       name=f"b{name}", bufs=1)
                    nc.gpsimd.dma_start(t[:], bdram[None, :])
                    b_row[name] = t
                # per-partition bias columns for q/k (dout on partitions)
                bcol = {}
                for name in ("q", "k"):
                    bc_t = consts.tile([128, NCH], f32, tag=f"bcol{name}",
                                       name=f"bcol{name}")
                    for c in range(NCH):
                        tp = ps512.tile([128, 512], f32, tag="ps512", name="ps512")
                        nc.tensor.transpose(
                            tp[:, 0:1], b_row[name][0:1, 128 * c:128 * (c + 1)],
                            ident[0:1, 0:1])
                        nc.vector.tensor_copy(bc_t[:, c:c + 1], tp[:, 0:1])
                    bcol[name] = bc_t
                # broadcast bias tiles for v (head-interleaved) and o (natural)
                bvb = consts.tile([128, H, DH], f32, tag="bvb", name="bvb")
                bob = consts.tile([128, 512], f32, tag="bob", name="bob")
                for dst, row in ((bvb, b_row["v"]), (bob, b_row["o"])):
                    rowr = wtmp_pool.tile([1, 512], f32, tag="browr",
                                          name="browr", bufs=2)
                    nc.vector.tensor_copy(r(rowr[:]), row[:])
                    tp = ps512.tile([128, 512], f32, tag="ps512", name="ps512")
                    nc.tensor.matmul(tp[:], r(ones[0:1, 0:128]), r(rowr[:]),
                                     start=True, stop=True)
                    if dst is bvb:
                        nc.vector.tensor_copy(
                            dst[:], tp[:].rearrange("p (h d) -> p h d", h=H))
                    else:
                        nc.vector.tensor_copy(dst[:], tp[:])

            # ---- persistent activations ----
            kt_sb = [persist.tile([128, L], f32, tag=f"KT{c}", name=f"KT{c}")
                     for c in range(NCH)]
            v_sb = [persist.tile([128, H, DH + 1], bf16, tag=f"V{t}",
                        name=f"V{t}") for t in range(NT)]
            stage = [persist.tile([128, L], f32, tag=f"stage{c}", name=f"stage{c}")
                     for c in range(NCH)]

            with (
                tc.tile_pool(name="xin", bufs=XIN) as xin_pool,
                tc.tile_pool(name="qtg", bufs=2) as qtg_pool,
                tc.tile_pool(name="xt", bufs=XT) as xt_pool,
                tc.tile_pool(name="pexp", bufs=PEXP) as p_pool,
                tc.tile_pool(name="norm", bufs=1) as norm_pool,
                tc.tile_pool(name="osb", bufs=OSB) as o_pool,
            ):
                def emit_a_pieces(g):
                    qt_g = [qtg_pool.tile([128, 512], f32, tag=f"qtg{c}",
                                          name=f"qtg{c}") for c in range(NCH)]
                    pieces = []
                    state = {}
                    for tname_, xdram_ in (("k", xk), ("v", xv), ("q", xq)):
                        pieces.append(
                            lambda tname=tname_, xdram=xdram_:
                            state.__setitem__(
                                tname, emit_a_transpose(g, xdram)))
                        pieces.append(
                            lambda tname=tname_: emit_a_proj(
                                g, tname, state[tname], qt_g))
                    return qt_g, pieces

                def emit_a_transpose(g, xdram):
                    if True:
                        xtiles = []
                        for j in range(4):
                            t0 = 4 * g + j
                            xt_in = xin_pool.tile([128, 512], f32, tag="xin",
                                                  name="xin")
                            nc.sync.dma_start(
                                xt_in[:], xdram[128 * t0:128 * (t0 + 1), :])
                            # cast to bf16 on the idle Pool engine so the
                            # transposes run at 1 c/row instead of 2
                            xb = xt_pool.tile([128, 512], bf16, tag="xb",
                                              name="xb", bufs=6)
                            nc.gpsimd.tensor_copy(xb[:], xt_in[:])
                            xtiles.append(xb)
                        xt_c = []
                        for c in range(NCH):
                            ps = ps512.tile([128, 512], f32, tag="ps512",
                                            name="ps512")
                            psb = ps[:, 0:256].bitcast(bf16)
                            for j in range(4):
                                nc.tensor.transpose(
                                    psb[:, 128 * j:128 * (j + 1)],
                                    xtiles[j][:, 128 * c:128 * (c + 1)],
                                    identb[:],
                                )
                            sb = xt_pool.tile([128, 512], bf16, tag="xt",
                                              name="xt")
                            nc.vector.tensor_copy(sb[:], psb)
                            xt_c.append(sb)
                        return xt_c

                def emit_a_proj(g, tname, xt_c, qt_g):
                    if True:
                        if tname in ("q", "k"):
                            for co in range(NCH):
                                pp = ps512.tile([128, 512], f32, tag="ps512",
                                                name="ps512")
                                for ci in range(NCH):
                                    nc.tensor.matmul(
                                        pp[:],
                                        w_sb[tname][
                                            :, ci, 128 * co:128 * (co + 1)],
                                        xt_c[ci][:],
                                        start=(ci == 0), stop=(ci == NCH - 1),
                                    )
                                if tname == "q":
                                    nc.vector.tensor_scalar_add(
                                        r(qt_g[co][:]), pp[:],
                                        bcol["q"][:, co:co + 1])
                                else:
                                    nc.vector.tensor_scalar_add(
                                        r(kt_sb[co][:, 512 * g:512 * (g + 1)]),
                                        pp[:], bcol["k"][:, co:co + 1])
                        else:
                            for j in range(4):
                                t0 = 4 * g + j
                                pv = ps512.tile([128, 512], f32, tag="ps512",
                                                name="ps512")
                                for ci in range(NCH):
                                    nc.tensor.matmul(
                                        pv[:],
                                        xt_c[ci][:, 128 * j:128 * (j + 1)],
                                        w_sb["v"][:, ci, :],
                                        start=(ci == 0), stop=(ci == NCH - 1),
                                    )
                                nc.vector.tensor_add(
                                    v_sb[t0][:, :, 0:DH],
                                    pv[:].rearrange("p (h d) -> p h d", h=H),
                                    bvb[:],
                                )
                                nc.gpsimd.memset(v_sb[t0][:, :, DH:DH + 1], 1.0)

                def emit_b_qt(qt, qt_g, weave=()):
                    weave = list(weave)
                    kmax = 4 * qt + 4
                    stg = norm_pool.tile([128, 1536], f32, tag="stg", name="stg")
                    for hp in range(H // 2):
                        ch = hp
                        kth = kt_sb[ch]
                        qth = qt_g[ch]
                        po = [ops_pool.tile([65, 512], f32, tag="ops",
                                            name="ops") for _ in range(2)]

                        def emit_av(c, pt):
                            m = c - 4 * qt
                            jv0 = 0 if m < 1 else 128 * m
                            for k in range(2):
                                nc.tensor.matmul(
                                    po[k][:, jv0:512],
                                    v_sb[c][:, 2 * hp + k, :],
                                    pt[:, k, jv0:512],
                                    start=(c == 0), stop=(c == kmax - 1),
                                )

                        pending = None
                        for c in range(kmax):
                            m = c - 4 * qt
                            js0 = 0 if m < 1 else (128 * m if m < 3 else 256)
                            jv0 = 0 if m < 1 else 128 * m
                            ps = sps2_pool.tile([128, 2, 512], f32,
                                                tag="sps2", name="sps2")
                            pt = p_pool.tile([128, 2, 512], bf16, tag="pexp",
                                             name="pexp")
                            for k in range(2):
                                prow = 64 * k
                                nc.tensor.matmul(
                                    ps[:, k, js0:512],
                                    r(kth[prow:prow + DH,
                                          128 * c:128 * (c + 1)]),
                                    r(qth[prow:prow + DH, js0:512]),
                                    start=True, stop=True,
                                )
                            if m < 0:
                                nc.scalar.activation(
                                    pt[:], ps[:],
                                    mybir.ActivationFunctionType.Exp,
                                    scale=0.125,
                                )
                            else:
                                # accumulate -240 above the diagonal of the
                                # 128-wide diag block (PE, no DVE hop)
                                for k in range(2):
                                    nc.tensor.matmul(
                                        ps[:, k, 128 * m:128 * (m + 1)],
                                        identb[:], maskU[:],
                                        start=False, stop=True,
                                        skip_group_check=True,
                                    )
                                # one strided-AP exp for both heads
                                nc.scalar.activation(
                                    pt[:, :, jv0:512], ps[:, :, jv0:512],
                                    mybir.ActivationFunctionType.Exp,
                                    scale=0.125,
                                )
                            if pending is not None:
                                emit_av(*pending)
                            pending = (c, pt)
                        emit_av(*pending)
                        for k in range(2):
                            h = 2 * hp + k
                            prow = 64 * k
                            nc.vector.tensor_copy(
                                r(stage[ch][prow:prow + DH,
                                            512 * qt:512 * (qt + 1)]),
                                po[k][0:DH, :])
                            nc.vector.tensor_copy(
                                stg[32 * (h % 3):32 * (h % 3) + 1,
                                    512 * (h // 3):512 * (h // 3) + 512],
                                po[k][DH:DH + 1, :])
                        if weave and hp >= 1:
                            weave.pop(0)()
                            if weave:
                                weave.pop(0)()
                    rstg = norm_pool.tile([128, 1536], f32, tag="rstg",
                                          name="rstg")
                    nc.vector.reciprocal_approx_fast(out=rstg[:], in_=stg[:])
                    rbf = norm_pool.tile([128, 1536], bf16, tag="rbf", name="rbf")
                    nc.vector.tensor_copy(rbf[:], rstg[:])
                    for ch in range(NCH):
                        bcp = ps512.tile([128, 512], f32, tag="ps512",
                                         name="ps512")
                        for sub in range(2):
                            hh = 2 * ch + sub
                            pp0 = 32 * (hh % 3)
                            fo = 512 * (hh // 3)
                            nc.tensor.matmul(
                                bcp[64 * sub:64 * sub + 64, :],
                                ones_t[pp0:pp0 + 1, 0:64],
                                rbf[pp0:pp0 + 1, fo:fo + 512],
                                start=True, stop=True,
                            )
                        nc.vector.tensor_mul(
                            r(stage[ch][:, 512 * qt:512 * (qt + 1)]),
                            stage[ch][:, 512 * qt:512 * (qt + 1)],
                            bcp[:],
                        )
                    for i in range(4 * qt, 4 * qt + 4):
                        pout = ps512.tile([128, 512], f32, tag="ps512",
                                          name="ps512")
                        for ch in range(NCH):
                            nc.tensor.matmul(
                                pout[:],
                                r(stage[ch][:, 128 * i:128 * (i + 1)]),
                                r(w_sb["o"][:, ch, :]),
                                start=(ch == 0), stop=(ch == NCH - 1),
                            )
                        ot = o_pool.tile([128, 512], f32, tag="osb", name="osb")
                        nc.vector.tensor_add(ot[:], pout[:], bob[:])
                        nc.sync.dma_start(out[128 * i:128 * (i + 1), :], ot[:])
                    for w in weave:
                        w()

                def emit_body():
                    qt_g, pieces = emit_a_pieces(0)
                    for p in pieces:
                        p()
                    for g in range(NQ):
                        if g + 1 < NQ:
                            qt_next, weave = emit_a_pieces(g + 1)
                        else:
                            qt_next, weave = None, ()
                        emit_b_qt(g, qt_g, weave)
                        qt_g = qt_next

                if repeat > 1:
                    with tc.For_i(0, repeat, 1, hint_engines=(
                            mybir.EngineType.PE,
                            mybir.EngineType.DVE,
                            mybir.EngineType.Activation,
                            mybir.EngineType.SP,
                            mybir.EngineType.Pool)):
                        emit_body()
                else:
                    emit_body()

    nc.compile()
    return nc


def get_nc(repeat=1, cfg=None):
    key = f"nc{repeat}-{sorted((cfg or {}).items())}"
    if key not in _cached:
        _cached[key] = _build(repeat, cfg)
    return _cached[key]


def run(in_maps, trace=False, repeat=1, cfg=None, **kw):
    from concourse.bass_utils import run_bass_kernel_spmd

    nc = get_nc(repeat, cfg)
    return run_bass_kernel_spmd(nc, in_maps, list(range(B)), trace=trace, **kw)


def kernel(query, key, value, Wq, bq, Wk, bk, Wv, bv, Wo, bo):
    shared = {
        "Wq": np.ascontiguousarray(Wq, np.float32),
        "Wk": np.ascontiguousarray(Wk, np.float32),
        "Wv": np.ascontiguousarray(Wv, np.float32),
        "Wo": np.ascontiguousarray(Wo, np.float32),
        "bq": np.ascontiguousarray(bq, np.float32),
        "bk": np.ascontiguousarray(bk, np.float32),
        "bv": np.ascontiguousarray(bv, np.float32),
        "bo": np.ascontiguousarray(bo, np.float32),
    }
    in_maps = []
    for i in range(B):
        m = dict(shared)
        m["query"] = np.ascontiguousarray(query[i], np.float32)
        m["key"] = np.ascontiguousarray(key[i], np.float32)
        m["value"] = np.ascontiguousarray(value[i], np.float32)
        in_maps.append(m)
    res = run(in_maps)
    return np.stack([res.results[i]["out"] for i in range(B)], axis=0)
